# revision 57
# baseline (speedup 1.0000x reference)
"""Bayesian dense MoE (top-2 of 8 experts) on 8 Trainium2 NeuronCores.

Math (per reference):
    logits = x @ gk + gb                      [B, E]
    gw     = renorm-top2(softmax(logits))     [B, E]   (softmax denom cancels)
    se     = softplus(rho) * eps              [U, E]
    out[b,u] = sum_e gw[b,e] * ( (x @ mu[:,:,e])[b,u] + s[b]*se[u,e] + bias[u,e] )
    with s[b] = sum_d x[b,d].

Sharding: data-parallel over batch. Each of the 8 cores processes 512 rows
of x and produces its 512-row slice of the output; the host concatenates.
No collectives needed.

The expert matmuls run in fp8e4 with DoubleRow perf mode (two 128-row
contraction groups per pass); gating runs in fp32r. Measured end-to-end
relative error vs a float64 reference: ~1.06e-2 (gate is 2e-2).

Structure notes:
 - x is uploaded twice: fp32 (d-major, for gating + row sums) and a
   host-side fp8e4 cast (for the expert matmuls).
 - noise+bias are folded into one matrix m = s*(gw@seT) + gw@biasT via
   PE matmuls of the s-scaled gates, so the DVE only does the 8-expert
   gate-weighted PSUM combines plus one final add per output tile.
 - ACT function tables (Exp/Ln/Copy) are prefetched during the initial
   DMA window so they don't serialize the gating chain.
"""

import numpy as np
import ml_dtypes

import concourse.bass as bass
from concourse import bacc
import concourse.mybir as mybir
import concourse.tile as tile
from concourse.bass_utils import run_bass_kernel_spmd
from concourse.masks import make_identity

N_CORES = 8
B, D, U, E = 4096, 1024, 1024, 8
P = 128                 # partitions
BS = B // N_CORES       # 512 batch rows per core
KT = D // P             # 8 contraction tiles
BT = BS // P            # 4 batch tiles per core
NT = 512                # matmul moving free dim (one PSUM bank of fp32)
UT = U // NT            # 2 output column tiles

F32 = mybir.dt.float32
F32R = mybir.dt.float32r
BF16 = mybir.dt.bfloat16
F8 = mybir.dt.float8e4
AF = mybir.ActivationFunctionType
ALU = mybir.AluOpType
DR = mybir.MatmulPerfMode.DoubleRow

_CACHE: dict = {}


def _emit(nc, tc, pools, inst, xT, x8, muR, gk, gb, rhoT, epsT, biasT, onesd, y):
    cp, pp, gt, wp, yp = pools
    if True:
        # Resident inputs.  DMA issue order matters: the first expert matmul
        # only needs x8 + w(ut0,e0), so those go first; xT (gating) follows.
        # DMA order is the startup critical path: the first expert matmul
        # needs x8+w00 (interleaved per kt-pair so PE starts after ~0.25MB);
        # the combine chain is bounded by xt (gating), so xt goes right after.
        x8_sb = cp.tile([P, KT, BS], F8)
        nc.sync.dma_start(out=x8_sb, in_=x8.rearrange("(kt p) b -> p kt b", p=P))
        w0_sb = cp.tile([P, KT, NT], F8)
        w0src = muR[0, :, 0:NT].rearrange("(kt p) n -> p kt n", p=P)
        nc.sync.dma_start(out=w0_sb[:, 0:KT // 2, :], in_=w0src[:, 0:KT // 2, :])
        nc.sync.dma_start(out=w0_sb[:, KT // 2:, :], in_=w0src[:, KT // 2:, :])
        gk_sb = cp.tile([P, KT, 10], F32R)
        nc.sync.dma_start(out=gk_sb, in_=gk.rearrange("(kt p) e -> p kt e", p=P))
        gb_sb = cp.tile([1, 10], F32R)
        nc.sync.dma_start(out=gb_sb, in_=gb)
        xt_sb = cp.tile([P, KT, BS], F32R)
        xv = xT.rearrange("(kt p) b -> p kt b", p=P)
        nc.sync.dma_start(out=xt_sb[:, 0:KT // 2, :], in_=xv[:, 0:KT // 2, :])
        nc.sync.dma_start(out=xt_sb[:, KT // 2:, :], in_=xv[:, KT // 2:, :])
        w1_sb = cp.tile([P, KT, NT], F8)
        w1src = muR[1, :, 0:NT].rearrange("(kt p) n -> p kt n", p=P)
        nc.sync.dma_start(out=w1_sb[:, 0:KT // 2, :], in_=w1src[:, 0:KT // 2, :])
        nc.sync.dma_start(out=w1_sb[:, KT // 2:, :], in_=w1src[:, KT // 2:, :])
        rho_sb = cp.tile([E, U], F32)
        nc.sync.dma_start(out=rho_sb, in_=rhoT)
        eps_sb = cp.tile([E, U], F32)
        nc.sync.dma_start(out=eps_sb, in_=epsT)
        bias_sb = cp.tile([E, U], F32)
        nc.sync.dma_start(out=bias_sb, in_=biasT)

        ones1 = cp.tile([1, P], F32R)
        nc.sync.dma_start(out=ones1, in_=onesd)
        ident = cp.tile([P, P], F32)
        make_identity(nc, ident)

        # Gating/noise intermediates kept for the whole kernel
        gw_sb = cp.tile([P, BT, E], F32)    # renormalized top-2 gates
        s_sb = cp.tile([P, BT], F32)        # per-row sums of x
        gwT_sb = cp.tile([E, BS], F32)      # gates transposed (e on partitions)
        csT_sb = cp.tile([E, BS], F32)      # (s*gw) transposed
        seT_sb = cp.tile([E, U], F32)       # softplus(rho)*eps, (e, u) layout

        if True:
            # ---- ACT function-table prefetch (hidden under the input DMAs) ----
            dmy = gt.tile([1, 2], F32, tag="dmy")
            nc.vector.memset(dmy, 0.0)
            dmy2 = gt.tile([1, 2], F32, tag="dmy2")
            nc.scalar.activation(out=dmy2, in_=dmy, func=AF.Exp)
            nc.scalar.activation(out=dmy2, in_=dmy2, func=AF.Ln, bias=1.0)
            nc.scalar.copy(dmy2, dmy)

            # ---- expert weight loader ----
            def load_w(ut, e):
                # per-body weight tag: body B's loads rotate independently of
                # body A's, so the cross-body weight pipeline has full slack
                # without deepening any single rotation
                w = wp.tile([P, KT, NT], F8, tag=f"w{inst}", bufs=3,
                            name=f"w_{inst}_{ut}_{e}")
                wsrc = muR[e, :, ut * NT:(ut + 1) * NT].rearrange(
                    "(kt p) n -> p kt n", p=P
                )
                nc.sync.dma_start(out=w[:, 0:KT // 2, :], in_=wsrc[:, 0:KT // 2, :])
                nc.sync.dma_start(out=w[:, KT // 2:, :], in_=wsrc[:, KT // 2:, :])
                return w

            # ---- gating: all 4 row-tiles in one wide [128, 4, 8] chain ----
            def gating_all():
                pg = pp.tile([P, BT, 10], F32, tag="gat", bufs=2, name=f"pg_all_{inst}")
                for bt in range(BT):
                    for kt in range(KT):
                        nc.tensor.matmul(
                            pg[:, bt, :],
                            lhsT=xt_sb[:, kt, bt * P:(bt + 1) * P],
                            rhs=gk_sb[:, kt, :],
                            start=(kt == 0),
                            stop=False,
                        )
                    # gating bias (and 0 for the row-sum column): ones^T x gb
                    nc.tensor.matmul(
                        pg[:, bt, :], lhsT=ones1, rhs=gb_sb, start=False, stop=True
                    )

                logit = pg[:, :, 0:8]                       # [128, 4, 8]
                m1 = gt.tile([P, BT, 1], F32, tag="m1")
                nc.vector.tensor_reduce(out=m1, in_=logit, axis=mybir.AxisListType.X, op=ALU.max)
                mask = gt.tile([P, BT, 8], F32, tag="mask")
                nc.vector.tensor_tensor(
                    out=mask, in0=logit, in1=m1.broadcast_to([P, BT, 8]), op=ALU.is_equal
                )
                l2 = gt.tile([P, BT, 8], F32, tag="l2")
                nc.vector.scalar_tensor_tensor(
                    out=l2, in0=mask, scalar=-1e30, in1=logit, op0=ALU.mult, op1=ALU.add
                )
                m2 = gt.tile([P, BT, 1], F32, tag="m2")
                nc.vector.tensor_reduce(out=m2, in_=l2, axis=mybir.AxisListType.X, op=ALU.max)
                nc.vector.tensor_tensor(
                    out=mask, in0=logit, in1=m2.broadcast_to([P, BT, 8]), op=ALU.is_ge
                )

                el = gt.tile([P, BT, 8], F32, tag="el")
                nc.scalar.activation(out=el, in_=logit, func=AF.Exp)
                gm = gt.tile([P, BT, 8], F32, tag="gm")
                nc.vector.tensor_mul(gm, el, mask)
                den = gt.tile([P, BT, 1], F32, tag="den")
                nc.vector.tensor_reduce(out=den, in_=gm, axis=mybir.AxisListType.X, op=ALU.add)
                inv = gt.tile([P, BT, 1], F32, tag="inv")
                nc.vector.reciprocal(inv, den)
                nc.vector.tensor_tensor(
                    out=gw_sb, in0=gm, in1=inv.broadcast_to([P, BT, 8]), op=ALU.mult
                )
                nc.scalar.copy(s_sb, pg[:, :, 8])
                # s-scaled gates for the combined noise+bias matrix
                cs_all = gt.tile([P, BT, 8], F32, tag="cs")
                nc.vector.tensor_tensor(
                    out=cs_all, in0=gw_sb,
                    in1=s_sb[:, :, None].broadcast_to([P, BT, 8]), op=ALU.mult
                )
                return cs_all

            # transpose gw and cs to (e, b) layout for the noise/bias matmuls
            def transposes(bt, cs):
                pt = pp.tile([8, P], F32, tag="gat", bufs=2)
                nc.tensor.transpose(pt, gw_sb[:, bt, :], ident)
                nc.scalar.copy(gwT_sb[:, bt * P:(bt + 1) * P], pt)
                pt2 = pp.tile([8, P], F32, tag="gat", bufs=2)
                nc.tensor.transpose(pt2, cs, ident)
                nc.scalar.copy(csT_sb[:, bt * P:(bt + 1) * P], pt2)

            # ya[bt] := m = s[b]*sum_e gw*se + sum_e gw*bias for one (ut, bt);
            # the expert combines then accumulate on top of it.
            def m_init(ut, bt, ya):
                mp = pp.tile([P, NT], F32, tag="gat", bufs=2)
                nc.tensor.matmul(
                    mp, lhsT=csT_sb[:, bt * P:(bt + 1) * P],
                    rhs=seT_sb[:, ut * NT:(ut + 1) * NT], start=True, stop=False,
                )
                nc.tensor.matmul(
                    mp, lhsT=gwT_sb[:, bt * P:(bt + 1) * P],
                    rhs=bias_sb[:, ut * NT:(ut + 1) * NT], start=False, stop=True,
                )
                # alternate engines so the 8 PSUM->SBUF inits don't serialize
                if bt % 2 == 0:
                    nc.scalar.copy(ya[bt], mp)
                else:
                    nc.vector.tensor_copy(ya[bt], mp)

            # ---- expert matmul group: 4 DoubleRow MMs per (e, bt) ----
            def expert_mms_only(w, ut, e):
                # allocate each PSUM tile right before its matmuls: at the
                # unrolled-body handoff the group starts on the first freed
                # bank instead of waiting for four
                ps = []
                for bt in range(BT):
                    t = pp.tile([P, NT], F32, tag="ps", bufs=6,
                                name=f"ps_{inst}_{ut}_{e}_{bt}")
                    ps.append(t)
                    for kt in range(0, KT, 2):
                        nc.tensor.matmul(
                            t,
                                lhsT=x8_sb[:, kt:kt + 2, bt * P:(bt + 1) * P],
                                rhs=w[:, kt:kt + 2, :],
                                start=(kt == 0), stop=(kt == KT - 2),
                                perf_mode=DR,
                            )
                return ps

            def combines(ps, ut, e, ya):
                for bt in range(BT):
                    gwe = gw_sb[:, bt, e:e + 1]
                    nc.vector.scalar_tensor_tensor(
                        out=ya[bt], in0=ps[bt], scalar=gwe, in1=ya[bt],
                        op0=ALU.mult, op1=ALU.add,
                    )

            def expert_mms(w, ut, e, ya):
                combines(expert_mms_only(w, ut, e), ut, e, ya)

            # ---- main schedule: expert group (0,0) first (its data lands
            # first), gating runs behind it, small PE jobs fill the cracks ----
            ya_all = {}
            for ut in range(UT):
                ya_all[ut] = [
                    yp.tile([P, NT], F32, tag=f"ya{inst}_{bt}", bufs=2,
                            name=f"ya_{inst}_{ut}_{bt}")
                    for bt in range(BT)
                ]

            # PE starts on (ut0, e0) immediately; gating runs behind it; the
            # combines only begin once gw + the m-initialized ya are ready.
            ps00 = expert_mms_only(w0_sb, 0, 0)
            cs_all = gating_all()
            # noise coefficients se = softplus(rho) * eps (after gating in
            # ACT program order so it doesn't block the gating Exp ops)
            nc.scalar.activation(out=seT_sb, in_=rho_sb, func=AF.Exp)
            nc.scalar.activation(out=seT_sb, in_=seT_sb, func=AF.Ln, bias=1.0)
            nc.gpsimd.tensor_mul(seT_sb, seT_sb, eps_sb)

            ps01 = expert_mms_only(w1_sb, 0, 1)
            wq = [load_w(0, 2)]
            for bt in range(BT):
                transposes(bt, cs_all[:, bt, :])
            for bt in range(BT):
                m_init(0, bt, ya_all[0])
            combines(ps00, 0, 0, ya_all[0])
            combines(ps01, 0, 1, ya_all[0])
            for bt in range(BT):
                m_init(1, bt, ya_all[1])

            for ut in range(UT):
                ya = ya_all[ut]
                for e in range(E):
                    if ut == 0 and e < 2:
                        continue  # issued above
                    w = wq.pop(0)
                    nxt = ut * E + e + 1
                    if nxt < UT * E:
                        wq.append(load_w(nxt // E, nxt % E))
                    expert_mms(w, ut, e, ya)
                # store: ya holds m + sum_e gw*expert after the e7 combine
                for bt in range(BT):
                    nc.sync.dma_start(
                        out=y[bt * P:(bt + 1) * P, ut * NT:(ut + 1) * NT], in_=ya[bt]
                    )


def build(reps=1):
    key = ("nc", reps)
    if key in _CACHE:
        return _CACHE[key]
    nc = bacc.Bacc("TRN2", target_bir_lowering=False)
    xT = nc.dram_tensor("xT", [D, BS], F32R, kind="ExternalInput").ap()
    x8 = nc.dram_tensor("x8", [D, BS], F8, kind="ExternalInput").ap()
    muR = nc.dram_tensor("muR", [E, D, U], F8, kind="ExternalInput").ap()
    gk = nc.dram_tensor("gk", [D, 10], F32R, kind="ExternalInput").ap()
    gb = nc.dram_tensor("gb", [1, 10], F32R, kind="ExternalInput").ap()
    rhoT = nc.dram_tensor("rhoT", [E, U], F32, kind="ExternalInput").ap()
    biasT = nc.dram_tensor("biasT", [E, U], F32, kind="ExternalInput").ap()
    epsT = nc.dram_tensor("epsT", [E, U], F32, kind="ExternalInput").ap()
    onesd = nc.dram_tensor("onesd", [1, P], F32R, kind="ExternalInput").ap()
    y = nc.dram_tensor("y", [BS, U], F32, kind="ExternalOutput").ap()
    args = (xT, x8, muR, gk, gb, rhoT, epsT, biasT, onesd, y)
    with tile.TileContext(nc) as tc:
        # Pools live at the top level and are shared by both unrolled loop
        # bodies: bufs=2 on the const pool double-buffers the per-iteration
        # input tiles, so body B's input DMAs overlap body A's compute.
        with (
            tc.tile_pool(name="const", bufs=2) as cp,
            tc.tile_pool(name="psum", bufs=1, space="PSUM") as pp,
            tc.tile_pool(name="gtmp", bufs=2) as gt,
            tc.tile_pool(name="wpool", bufs=3) as wp,
            tc.tile_pool(name="ypool", bufs=2) as yp,
        ):
            pools = (cp, pp, gt, wp, yp)
            if reps == 1:
                _emit(nc, tc, pools, 0, *args)
            else:
                assert reps % 2 == 0, "looped builds must have even reps"
                with tc.For_i(0, reps, 2):
                    _emit(nc, tc, pools, 0, *args)
                    _emit(nc, tc, pools, 1, *args)
    nc.compile()
    _CACHE[key] = nc
    return nc


def prep_inputs(x, expert_mu, expert_rho, expert_bias, gating_kernel, gating_bias, eps):
    """Host-side sharding / layout prep (no math beyond dtype rounding)."""
    x = np.ascontiguousarray(np.asarray(x, dtype=np.float32))
    mu = np.asarray(expert_mu, dtype=np.float32)        # [D, U, E]
    bias = np.asarray(expert_bias, dtype=np.float32)    # [U, E]
    # e-major weights, fp8e4 for the DoubleRow matmuls
    muR = np.ascontiguousarray(
        np.transpose(mu, (2, 0, 1)).astype(ml_dtypes.float8_e4m3)
    )
    gk = np.concatenate(
        [np.asarray(gating_kernel, dtype=np.float32), np.ones((D, 1), np.float32),
         np.zeros((D, 1), np.float32)], axis=1
    )  # [D, 10]: col 8 computes the row-sums s; col 9 pads to even width (fp32r ISA)
    gb = np.concatenate(
        [np.asarray(gating_bias, dtype=np.float32), np.zeros((2,), np.float32)]
    ).reshape(1, 10)
    rhoT = np.ascontiguousarray(np.asarray(expert_rho, dtype=np.float32).T)  # [E, U]
    epsT = np.ascontiguousarray(np.asarray(eps, dtype=np.float32).T)         # [E, U]
    biasT = np.ascontiguousarray(bias.T)                                     # [E, U]
    shared = {"muR": muR, "gk": gk, "gb": gb, "rhoT": rhoT, "epsT": epsT, "biasT": biasT,
              "onesd": np.ones((1, P), np.float32)}
    in_maps = []
    for c in range(N_CORES):
        xs = np.ascontiguousarray(x[c * BS:(c + 1) * BS].T)  # [D, BS]
        x8s = np.ascontiguousarray(xs.astype(ml_dtypes.float8_e4m3))
        in_maps.append({"xT": xs, "x8": x8s, **shared})
    return in_maps


def kernel(x, expert_mu, expert_rho, expert_bias, gating_kernel, gating_bias, eps, k):
    assert int(k) == 2, f"kernel is specialized for top-2 gating, got k={k}"
    nc = build()
    in_maps = prep_inputs(
        x, expert_mu, expert_rho, expert_bias, gating_kernel, gating_bias, eps
    )
    res = run_bass_kernel_spmd(nc, in_maps, list(range(N_CORES)))
    return np.concatenate([res.results[c]["y"] for c in range(N_CORES)], axis=0)


# revision 60
# speedup vs baseline: 1.0250x; 1.0250x over previous
"""Bayesian dense MoE (top-2 of 8 experts) on 8 Trainium2 NeuronCores.

Math (per reference):
    logits = x @ gk + gb                      [B, E]
    gw     = renorm-top2(softmax(logits))     [B, E]   (softmax denom cancels)
    se     = softplus(rho) * eps              [U, E]
    out[b,u] = sum_e gw[b,e] * ( (x @ mu[:,:,e])[b,u] + s[b]*se[u,e] + bias[u,e] )
    with s[b] = sum_d x[b,d].

Sharding: data-parallel over batch. Each of the 8 cores processes 512 rows
of x and produces its 512-row slice of the output; the host concatenates.
No collectives needed.

The expert matmuls run in fp8e4 with DoubleRow perf mode (two 128-row
contraction groups per pass); gating runs in fp32r (top-2 selection is
precision-critical). Measured end-to-end relative error vs a float64
reference: 1.087e-2 (gate is 2e-2).

Looped builds unroll the body x2 with double-buffered input tiles and
per-body weight-buffer tags, so iteration N+1's DMA-bound startup
pipelines under iteration N's compute tail.

Structure notes:
 - x is uploaded twice: fp32 (d-major, for gating + row sums) and a
   host-side fp8e4 cast (for the expert matmuls).
 - noise+bias are folded into one matrix m = s*(gw@seT) + gw@biasT via
   PE matmuls of the s-scaled gates, so the DVE only does the 8-expert
   gate-weighted PSUM combines plus one final add per output tile.
 - ACT function tables (Exp/Ln/Copy) are prefetched during the initial
   DMA window so they don't serialize the gating chain.
"""

import numpy as np
import ml_dtypes

import concourse.bass as bass
from concourse import bacc
import concourse.mybir as mybir
import concourse.tile as tile
from concourse.bass_utils import run_bass_kernel_spmd
from concourse.masks import make_identity

N_CORES = 8
B, D, U, E = 4096, 1024, 1024, 8
P = 128                 # partitions
BS = B // N_CORES       # 512 batch rows per core
KT = D // P             # 8 contraction tiles
BT = BS // P            # 4 batch tiles per core
NT = 512                # matmul moving free dim (one PSUM bank of fp32)
UT = U // NT            # 2 output column tiles

F32 = mybir.dt.float32
F32R = mybir.dt.float32r
BF16 = mybir.dt.bfloat16
F8 = mybir.dt.float8e4
AF = mybir.ActivationFunctionType
ALU = mybir.AluOpType
DR = mybir.MatmulPerfMode.DoubleRow

_CACHE: dict = {}


def _emit(nc, tc, pools, inst, xT, x8, muR, gk, gb, rhoT, epsT, biasT, onesd, y):
    cp, pp, gt, wp, yp = pools
    if True:
        # Resident inputs.  DMA issue order is the startup critical path:
        # the first expert matmul only needs x8 + w(ut0,e0), so those go
        # first; xT (gating) follows because the combine chain is bounded
        # by its arrival.
        x8_sb = cp.tile([P, KT, BS], F8)
        nc.sync.dma_start(out=x8_sb, in_=x8.rearrange("(kt p) b -> p kt b", p=P))
        w0_sb = cp.tile([P, KT, NT], F8)
        w0src = muR[0, :, 0:NT].rearrange("(kt p) n -> p kt n", p=P)
        nc.sync.dma_start(out=w0_sb[:, 0:KT // 2, :], in_=w0src[:, 0:KT // 2, :])
        nc.sync.dma_start(out=w0_sb[:, KT // 2:, :], in_=w0src[:, KT // 2:, :])
        gk_sb = cp.tile([P, KT, 10], F32R)
        nc.sync.dma_start(out=gk_sb, in_=gk.rearrange("(kt p) e -> p kt e", p=P))
        gb_sb = cp.tile([1, 10], F32R)
        nc.sync.dma_start(out=gb_sb, in_=gb)
        xt_sb = cp.tile([P, KT, BS], F32R)
        xv = xT.rearrange("(kt p) b -> p kt b", p=P)
        nc.sync.dma_start(out=xt_sb[:, 0:KT // 2, :], in_=xv[:, 0:KT // 2, :])
        nc.sync.dma_start(out=xt_sb[:, KT // 2:, :], in_=xv[:, KT // 2:, :])
        w1_sb = cp.tile([P, KT, NT], F8)
        w1src = muR[1, :, 0:NT].rearrange("(kt p) n -> p kt n", p=P)
        nc.sync.dma_start(out=w1_sb[:, 0:KT // 2, :], in_=w1src[:, 0:KT // 2, :])
        nc.sync.dma_start(out=w1_sb[:, KT // 2:, :], in_=w1src[:, KT // 2:, :])
        rho_sb = cp.tile([E, U], F32)
        nc.sync.dma_start(out=rho_sb, in_=rhoT)
        eps_sb = cp.tile([E, U], F32)
        nc.sync.dma_start(out=eps_sb, in_=epsT)
        bias_sb = cp.tile([E, U], F32)
        nc.sync.dma_start(out=bias_sb, in_=biasT)

        ones1 = cp.tile([1, P], F32R)
        nc.sync.dma_start(out=ones1, in_=onesd)
        ident = cp.tile([P, P], F32)
        make_identity(nc, ident)

        # Gating/noise intermediates kept for the whole kernel
        gw_sb = cp.tile([P, BT, E], F32)    # renormalized top-2 gates
        s_sb = cp.tile([P, BT], F32)        # per-row sums of x
        gwT_sb = cp.tile([E, BS], F32)      # gates transposed (e on partitions)
        csT_sb = cp.tile([E, BS], F32)      # (s*gw) transposed
        seT_sb = cp.tile([E, U], F32)       # softplus(rho)*eps, (e, u) layout

        if True:
            # ---- ACT function-table prefetch (hidden under the input DMAs) ----
            dmy = gt.tile([1, 2], F32, tag="dmy")
            nc.vector.memset(dmy, 0.0)
            dmy2 = gt.tile([1, 2], F32, tag="dmy2")
            nc.scalar.activation(out=dmy2, in_=dmy, func=AF.Exp)
            nc.scalar.activation(out=dmy2, in_=dmy2, func=AF.Ln, bias=1.0)
            nc.scalar.copy(dmy2, dmy)

            # ---- expert weight loader ----
            def load_w(ut, e):
                # per-body weight tag: body B's loads rotate independently of
                # body A's, so the cross-body weight pipeline has full slack
                # without deepening any single rotation
                w = wp.tile([P, KT, NT], F8, tag=f"w{inst}", bufs=3,
                            name=f"w_{inst}_{ut}_{e}")
                wsrc = muR[e, :, ut * NT:(ut + 1) * NT].rearrange(
                    "(kt p) n -> p kt n", p=P
                )
                nc.sync.dma_start(out=w[:, 0:KT // 2, :], in_=wsrc[:, 0:KT // 2, :])
                nc.sync.dma_start(out=w[:, KT // 2:, :], in_=wsrc[:, KT // 2:, :])
                return w

            # ---- gating: all 4 row-tiles in one wide [128, 4, 8] chain ----
            def gating_all():
                pg = pp.tile([P, BT, 10], F32, tag="gat", bufs=2, name=f"pg_all_{inst}")
                for bt in range(BT):
                    for kt in range(KT):
                        nc.tensor.matmul(
                            pg[:, bt, :],
                            lhsT=xt_sb[:, kt, bt * P:(bt + 1) * P],
                            rhs=gk_sb[:, kt, :],
                            start=(kt == 0),
                            stop=False,
                        )
                    # gating bias (and 0 for the row-sum column): ones^T x gb
                    nc.tensor.matmul(
                        pg[:, bt, :], lhsT=ones1, rhs=gb_sb, start=False, stop=True
                    )

                logit = pg[:, :, 0:8]                       # [128, 4, 8]
                m1 = gt.tile([P, BT, 1], F32, tag="m1")
                nc.vector.tensor_reduce(out=m1, in_=logit, axis=mybir.AxisListType.X, op=ALU.max)
                mask = gt.tile([P, BT, 8], F32, tag="mask")
                nc.vector.tensor_tensor(
                    out=mask, in0=logit, in1=m1.broadcast_to([P, BT, 8]), op=ALU.is_equal
                )
                l2 = gt.tile([P, BT, 8], F32, tag="l2")
                nc.vector.scalar_tensor_tensor(
                    out=l2, in0=mask, scalar=-1e30, in1=logit, op0=ALU.mult, op1=ALU.add
                )
                m2 = gt.tile([P, BT, 1], F32, tag="m2")
                nc.vector.tensor_reduce(out=m2, in_=l2, axis=mybir.AxisListType.X, op=ALU.max)
                nc.vector.tensor_tensor(
                    out=mask, in0=logit, in1=m2.broadcast_to([P, BT, 8]), op=ALU.is_ge
                )

                el = gt.tile([P, BT, 8], F32, tag="el")
                nc.scalar.activation(out=el, in_=logit, func=AF.Exp)
                gm = gt.tile([P, BT, 8], F32, tag="gm")
                nc.vector.tensor_mul(gm, el, mask)
                den = gt.tile([P, BT, 1], F32, tag="den")
                nc.vector.tensor_reduce(out=den, in_=gm, axis=mybir.AxisListType.X, op=ALU.add)
                inv = gt.tile([P, BT, 1], F32, tag="inv")
                nc.vector.reciprocal(inv, den)
                nc.vector.tensor_tensor(
                    out=gw_sb, in0=gm, in1=inv.broadcast_to([P, BT, 8]), op=ALU.mult
                )
                nc.scalar.copy(s_sb, pg[:, :, 8])
                # s-scaled gates for the combined noise+bias matrix
                cs_all = gt.tile([P, BT, 8], F32, tag="cs")
                nc.vector.tensor_tensor(
                    out=cs_all, in0=gw_sb,
                    in1=s_sb[:, :, None].broadcast_to([P, BT, 8]), op=ALU.mult
                )
                return cs_all

            # transpose gw and cs to (e, b) layout for the noise/bias matmuls
            def transposes(bt, cs):
                pt = pp.tile([8, P], F32, tag="gat", bufs=2)
                nc.tensor.transpose(pt, gw_sb[:, bt, :], ident)
                nc.scalar.copy(gwT_sb[:, bt * P:(bt + 1) * P], pt)
                pt2 = pp.tile([8, P], F32, tag="gat", bufs=2)
                nc.tensor.transpose(pt2, cs, ident)
                nc.scalar.copy(csT_sb[:, bt * P:(bt + 1) * P], pt2)

            # ya[bt] := m = s[b]*sum_e gw*se + sum_e gw*bias for one (ut, bt);
            # the expert combines then accumulate on top of it.
            def m_init(ut, bt, ya):
                mp = pp.tile([P, NT], F32, tag="gat", bufs=2)
                nc.tensor.matmul(
                    mp, lhsT=csT_sb[:, bt * P:(bt + 1) * P],
                    rhs=seT_sb[:, ut * NT:(ut + 1) * NT], start=True, stop=False,
                )
                nc.tensor.matmul(
                    mp, lhsT=gwT_sb[:, bt * P:(bt + 1) * P],
                    rhs=bias_sb[:, ut * NT:(ut + 1) * NT], start=False, stop=True,
                )
                # alternate engines so the 8 PSUM->SBUF inits don't serialize
                if bt % 2 == 0:
                    nc.scalar.copy(ya[bt], mp)
                else:
                    nc.vector.tensor_copy(ya[bt], mp)

            # ---- expert matmul group: 4 DoubleRow MMs per (e, bt) ----
            def expert_mms_only(w, ut, e, kt_major=False):
                ps = [
                    pp.tile([P, NT], F32, tag="ps", bufs=6, name=f"ps_{inst}_{ut}_{e}_{bt}")
                    for bt in range(BT)
                ]
                if kt_major:
                    # issue per kt-pair across all bt so compute starts as
                    # soon as the first x8/w chunk lands
                    for kt in range(0, KT, 2):
                        for bt in range(BT):
                            nc.tensor.matmul(
                                ps[bt],
                                lhsT=x8_sb[:, kt:kt + 2, bt * P:(bt + 1) * P],
                                rhs=w[:, kt:kt + 2, :],
                                start=(kt == 0), stop=(kt == KT - 2),
                                perf_mode=DR,
                            )
                else:
                    for bt in range(BT):
                        for kt in range(0, KT, 2):
                            nc.tensor.matmul(
                                ps[bt],
                                lhsT=x8_sb[:, kt:kt + 2, bt * P:(bt + 1) * P],
                                rhs=w[:, kt:kt + 2, :],
                                start=(kt == 0), stop=(kt == KT - 2),
                                perf_mode=DR,
                            )
                return ps

            def combines(ps, ut, e, ya):
                for bt in range(BT):
                    gwe = gw_sb[:, bt, e:e + 1]
                    nc.vector.scalar_tensor_tensor(
                        out=ya[bt], in0=ps[bt], scalar=gwe, in1=ya[bt],
                        op0=ALU.mult, op1=ALU.add,
                    )

            def expert_mms(w, ut, e, ya):
                combines(expert_mms_only(w, ut, e), ut, e, ya)

            # ---- main schedule: expert group (0,0) first (its data lands
            # first), gating runs behind it, small PE jobs fill the cracks ----
            ya_all = {}
            for ut in range(UT):
                ya_all[ut] = [
                    yp.tile([P, NT], F32, tag=f"ya{bt}", bufs=3, name=f"ya_{inst}_{ut}_{bt}")
                    for bt in range(BT)
                ]

            # PE starts on (ut0, e0) immediately; gating runs behind it; the
            # combines only begin once gw + the m-initialized ya are ready.
            ps00 = expert_mms_only(w0_sb, 0, 0)
            cs_all = gating_all()
            # noise coefficients se = softplus(rho) * eps (after gating in
            # ACT program order so it doesn't block the gating Exp ops)
            nc.scalar.activation(out=seT_sb, in_=rho_sb, func=AF.Exp)
            nc.scalar.activation(out=seT_sb, in_=seT_sb, func=AF.Ln, bias=1.0)
            nc.gpsimd.tensor_mul(seT_sb, seT_sb, eps_sb)

            ps01 = expert_mms_only(w1_sb, 0, 1)
            wq = [load_w(0, 2)]
            for bt in range(BT):
                transposes(bt, cs_all[:, bt, :])
            for bt in range(BT):
                m_init(0, bt, ya_all[0])
            combines(ps00, 0, 0, ya_all[0])
            combines(ps01, 0, 1, ya_all[0])
            for bt in range(BT):
                m_init(1, bt, ya_all[1])

            for ut in range(UT):
                ya = ya_all[ut]
                for e in range(E):
                    if ut == 0 and e < 2:
                        continue  # issued above
                    w = wq.pop(0)
                    nxt = ut * E + e + 1
                    if nxt < UT * E:
                        wq.append(load_w(nxt // E, nxt % E))
                    expert_mms(w, ut, e, ya)
                # store: ya holds m + sum_e gw*expert after the e7 combine
                for bt in range(BT):
                    nc.sync.dma_start(
                        out=y[bt * P:(bt + 1) * P, ut * NT:(ut + 1) * NT], in_=ya[bt]
                    )


def build(reps=1):
    key = ("nc", reps)
    if key in _CACHE:
        return _CACHE[key]
    nc = bacc.Bacc("TRN2", target_bir_lowering=False)
    xT = nc.dram_tensor("xT", [D, BS], F32R, kind="ExternalInput").ap()
    x8 = nc.dram_tensor("x8", [D, BS], F8, kind="ExternalInput").ap()
    muR = nc.dram_tensor("muR", [E, D, U], F8, kind="ExternalInput").ap()
    gk = nc.dram_tensor("gk", [D, 10], F32R, kind="ExternalInput").ap()
    gb = nc.dram_tensor("gb", [1, 10], F32R, kind="ExternalInput").ap()
    rhoT = nc.dram_tensor("rhoT", [E, U], F32, kind="ExternalInput").ap()
    biasT = nc.dram_tensor("biasT", [E, U], F32, kind="ExternalInput").ap()
    epsT = nc.dram_tensor("epsT", [E, U], F32, kind="ExternalInput").ap()
    onesd = nc.dram_tensor("onesd", [1, P], F32R, kind="ExternalInput").ap()
    y = nc.dram_tensor("y", [BS, U], F32, kind="ExternalOutput").ap()
    args = (xT, x8, muR, gk, gb, rhoT, epsT, biasT, onesd, y)
    with tile.TileContext(nc) as tc:
        # Pools live at the top level and are shared by both unrolled loop
        # bodies: bufs=2 on the const pool double-buffers the per-iteration
        # input tiles, so body B's input DMAs overlap body A's compute.
        with (
            tc.tile_pool(name="const", bufs=2) as cp,
            tc.tile_pool(name="psum", bufs=1, space="PSUM") as pp,
            tc.tile_pool(name="gtmp", bufs=2) as gt,
            tc.tile_pool(name="wpool", bufs=3) as wp,
            tc.tile_pool(name="ypool", bufs=2) as yp,
        ):
            pools = (cp, pp, gt, wp, yp)
            if reps == 1:
                _emit(nc, tc, pools, 0, *args)
            else:
                assert reps % 2 == 0, "looped builds must have even reps"
                with tc.For_i(0, reps, 2):
                    _emit(nc, tc, pools, 0, *args)
                    _emit(nc, tc, pools, 1, *args)
    nc.compile()
    _CACHE[key] = nc
    return nc


def prep_inputs(x, expert_mu, expert_rho, expert_bias, gating_kernel, gating_bias, eps):
    """Host-side sharding / layout prep (no math beyond dtype rounding)."""
    x = np.ascontiguousarray(np.asarray(x, dtype=np.float32))
    mu = np.asarray(expert_mu, dtype=np.float32)        # [D, U, E]
    bias = np.asarray(expert_bias, dtype=np.float32)    # [U, E]
    # e-major weights, fp8e4 for the DoubleRow matmuls
    muR = np.ascontiguousarray(
        np.transpose(mu, (2, 0, 1)).astype(ml_dtypes.float8_e4m3)
    )
    gk = np.concatenate(
        [np.asarray(gating_kernel, dtype=np.float32), np.ones((D, 1), np.float32),
         np.zeros((D, 1), np.float32)], axis=1
    )  # [D, 10]: col 8 computes the row-sums s; col 9 pads to even width (fp32r ISA)
    gb = np.concatenate(
        [np.asarray(gating_bias, dtype=np.float32), np.zeros((2,), np.float32)]
    ).reshape(1, 10)
    rhoT = np.ascontiguousarray(np.asarray(expert_rho, dtype=np.float32).T)  # [E, U]
    epsT = np.ascontiguousarray(np.asarray(eps, dtype=np.float32).T)         # [E, U]
    biasT = np.ascontiguousarray(bias.T)                                     # [E, U]
    shared = {"muR": muR, "gk": gk, "gb": gb, "rhoT": rhoT, "epsT": epsT, "biasT": biasT,
              "onesd": np.ones((1, P), np.float32)}
    in_maps = []
    for c in range(N_CORES):
        xs = np.ascontiguousarray(x[c * BS:(c + 1) * BS].T)  # [D, BS]
        x8s = np.ascontiguousarray(xs.astype(ml_dtypes.float8_e4m3))
        in_maps.append({"xT": xs, "x8": x8s, **shared})
    return in_maps


def kernel(x, expert_mu, expert_rho, expert_bias, gating_kernel, gating_bias, eps, k):
    assert int(k) == 2, f"kernel is specialized for top-2 gating, got k={k}"
    nc = build()
    in_maps = prep_inputs(
        x, expert_mu, expert_rho, expert_bias, gating_kernel, gating_bias, eps
    )
    res = run_bass_kernel_spmd(nc, in_maps, list(range(N_CORES)))
    return np.concatenate([res.results[c]["y"] for c in range(N_CORES)], axis=0)
